# revision 63
# baseline (speedup 1.0000x reference)
"""Location-sensitive attention Trainium2 kernel (v2).

Strategy (data-parallel over batch, 8 cores, B=128 -> 16 per core):
  - Encoder shipped in BOTH layouts as bf16 (transposed [E, B*T] for the
    projection matmul, natural [B*T, E] for the context matmul); the
    17MB of encoder DMA at ~350GB/s is the critical path, everything
    else overlaps it.
  - All small weights packed into ONE partition-major DRAM blob so the
    weight load is a single large-descriptor DMA (the old per-weight
    strided DMAs burned ~10us in 64-512B descriptors).
  - encT DMAs sliced bt-major (2 proj tiles per DMA) so the projection
    chases the stream instead of waiting for whole e-chunks.
  - conv1d folded into W_loc on the host; loc_p accumulates into the
    same PSUM tile as the encoder projection via a host im2col.
  - decoder projection applied as a per-partition bias in the tanh.
  - energies via per-batch column-masked W_e ("diag" trick): all 16
    tiles accumulate into one [16, 512] PSUM tile (softmax layout).
  - context phase wave-packed: 8 batches per [128, 512] PSUM tile
    (matmul out at partition offset 16*k), one PSUM->SBUF copy and one
    stride-17 row-gather DMA per wave instead of 16 tiny copies+DMAs.
  - outputs go out on the second HWDGE ring (scalar) so they don't
    queue behind the enc_nat input stream on the sync ring.
  - b_e dropped: softmax is shift-invariant.
"""

import sys

for p in ("/opt/trn_rl_repo",):
    if p not in sys.path:
        sys.path.insert(0, p)

import numpy as np
import ml_dtypes

import concourse.bass as bass
import concourse.tile as tile
from concourse import mybir
from concourse import bacc
from concourse import bass_utils
from concourse.masks import make_identity

BF = ml_dtypes.bfloat16

NCORES = 8
B, T, E, D, A, F, KW = 128, 512, 512, 1024, 128, 32, 31
BS = B // NCORES          # 16 batches per core
NT = BS                   # 16 bt-tiles of 512 (tile i == batch i)
P = 128


def build_device_program(nc):
    dt = mybir.dt
    f32, bf16 = dt.float32, dt.bfloat16
    Act = mybir.ActivationFunctionType

    # Partition-major layouts so every DMA descriptor is a large
    # contiguous run.
    wblob = nc.dram_tensor("wblob", (P, 20, 128), bf16, kind="ExternalInput").ap()
    prevrep = nc.dram_tensor("prevrep", (32, BS * T), dt.float8e4, kind="ExternalInput").ap()
    # slice-major: [p, slice, et, bt_within] so each slice DMA is one
    # contiguous 16KB run per partition (fat descriptors, fast DGE).
    # Exactly 8 input DMAs total: Tile has 8 DMA-completion semaphore
    # lanes; more DMAs means lane sharing, which entangles projection
    # waits with enc_nat/output completions.
    encT = nc.dram_tensor("encT", (P, 4, 4, BS * T // 4), bf16, kind="ExternalInput").ap()
    enc_nat = nc.dram_tensor("enc_nat", (P, BS * T // P, E), dt.float8e4, kind="ExternalInput").ap()
    ctx_out = nc.dram_tensor("context_out", (BS, E), f32, kind="ExternalOutput").ap()
    attn_out = nc.dram_tensor("attn_out", (BS, T), f32, kind="ExternalOutput").ap()

    with tile.TileContext(nc) as tc:
        with (
            tc.tile_pool(name="const", bufs=1) as const,
            tc.tile_pool(name="big", bufs=1) as big,
            tc.tile_pool(name="work", bufs=1) as work,
            tc.tile_pool(name="ps_pe", bufs=4, space="PSUM") as ps_pe,
            tc.tile_pool(name="ps_one", bufs=1, space="PSUM") as ps_one,
            tc.tile_pool(name="ps_sm", bufs=1, space="PSUM") as ps_sm,
            tc.tile_pool(name="ps_ctx", bufs=2, space="PSUM") as ps_ctx,
        ):
            # ---- DMA issue order on the sync ring = priority order ----
            wblob_sb = const.tile([P, 20, 128], bf16)
            nc.sync.dma_start(wblob_sb, wblob)
            encT_sb = big.tile([P, 4, 4, BS * T // 4], bf16)
            prevrep_sb = const.tile([32, BS * T], dt.float8e4)
            for s in range(4):
                nc.sync.dma_start(encT_sb[:, s], encT[:, s])
                if s == 0:
                    # prevrep is first needed by tile 0's conv matmul, just
                    # after encT slice 0 — slot it behind slice 0
                    nc.sync.dma_start(prevrep_sb, prevrep)
            enc_nat_sb = big.tile([P, BS * T // P, E], dt.float8e4)
            for h in range(2):
                nc.sync.dma_start(
                    enc_nat_sb[:, h * 32:(h + 1) * 32, :], enc_nat[:, h * 32:(h + 1) * 32, :]
                )

            ident16 = const.tile([16, 16], f32)
            make_identity(nc, ident16)

            # ---- PE warm-up: the HAM clock gate only un-throttles the PE
            # (1.2 -> 2.4 GHz) after ~3.4us of sustained full-array matmul
            # activity. The real matmul stream is DMA-paced with gaps, so
            # it never warms on its own. Run full-width dummy matmuls on a
            # zeroed tile while the PE waits for the first encT slice.
            dummy_sb = const.tile([P, T], bf16)
            nc.gpsimd.memset(dummy_sb, 0.0)
            warm_ps = ps_ctx.tile([P, T], f32, tag="ctx")
            for wu in range(52):
                nc.tensor.matmul(
                    warm_ps, lhsT=dummy_sb[:, 0:P], rhs=dummy_sb,
                    start=True, stop=True,
                )
            # pre-load the scalar engine's tanh/exp tables now, so the
            # table-load DMAs don't interleave with the encoder stream
            actwarm = work.tile([16, 16], bf16)
            nc.scalar.activation(actwarm, dummy_sb[0:16, 0:16], Act.Tanh)
            actwarm2 = work.tile([16, 16], bf16)
            nc.scalar.activation(
                actwarm2, dummy_sb[0:16, 0:16], Act.Exp, accum_out=None
            )

            # ---- projection + tanh + energies (chases the encT stream) ----
            # The energy matmul for tile i is emitted after tile i+1's
            # projection matmuls: the PE queue is in-order, so putting it
            # right after tile i's would stall the PE on tanh_i (scalar).
            psum_energ = ps_one.tile([BS, T], f32, tag="energ")
            tanh_tiles = []

            def energy_mm(i):
                eblk, ecol = 14 + i // 8, (i % 8) * 16
                nc.tensor.matmul(
                    psum_energ,
                    lhsT=wblob_sb[:, eblk, ecol:ecol + 16],
                    rhs=tanh_tiles[i],
                    start=(i == 0),
                    stop=(i == NT - 1),
                )

            decp_sb = const.tile([P, BS], f32)
            for i in range(NT):  # tile i == batch i
                pe_t = ps_pe.tile([A, T], f32, tag="pe")
                s, hh = i // 4, (i % 4) * T
                for et in range(4):
                    nc.tensor.matmul(
                        pe_t,
                        lhsT=wblob_sb[:, et, :],
                        rhs=encT_sb[:, s, et, hh:hh + T],
                        start=(et == 0),
                        stop=False,
                    )
                nc.tensor.matmul(
                    pe_t,
                    lhsT=wblob_sb[0:32, 13, :],
                    rhs=prevrep_sb[:, i * T:(i + 1) * T],
                    start=False,
                    stop=True,
                )
                if i == 0:
                    # dec_p (tanh bias): emitted after tile 0's projection
                    # so its wblob wait doesn't delay the first real matmul
                    psum_dec = ps_sm.tile([P, BS], f32, tag="small16")
                    for dti in range(8):
                        nc.tensor.matmul(
                            psum_dec,
                            lhsT=wblob_sb[:, 4 + dti, :],
                            rhs=wblob_sb[:, 12, dti * 16:(dti + 1) * 16],
                            start=(dti == 0),
                            stop=(dti == 7),
                        )
                    nc.vector.tensor_copy(decp_sb, psum_dec)
                tanh_t = work.tile([A, T], bf16, tag="tanh", bufs=4)
                nc.scalar.activation(
                    tanh_t, pe_t, Act.Tanh, bias=decp_sb[:, i:i + 1], scale=1.0
                )
                tanh_tiles.append(tanh_t)
                if i >= 1:
                    energy_mm(i - 1)
            energy_mm(NT - 1)

            # keep the PE's HAM clock-gate warm across the softmax gap
            warm2 = ps_pe.tile([P, T], f32, tag="pe")
            for wu in range(6):
                nc.tensor.matmul(
                    warm2, lhsT=dummy_sb[:, 0:P], rhs=dummy_sb,
                    start=True, stop=True,
                )

            # ---- softmax over T (psum_energ is [16, 512]) ----
            # no max-subtraction: |energies| <= ~9 (W_e is 1/sqrt(A)-scaled,
            # tanh in [-1,1]), so f32 exp cannot overflow
            attn_exp = work.tile([BS, T], f32)
            esum = work.tile([BS, 1], f32)
            nc.scalar.activation(
                attn_exp, psum_energ, Act.Exp, scale=1.0, accum_out=esum
            )
            rs = work.tile([BS, 1], f32)
            nc.vector.reciprocal(rs, esum)
            attn_f32 = work.tile([BS, T], f32)
            nc.vector.tensor_scalar_mul(attn_f32, attn_exp, rs)
            # output on the scalar HWDGE ring: doesn't queue behind enc_nat
            nc.scalar.dma_start(attn_out, attn_f32)

            # ---- transpose attn -> [t, b] columns ----
            # padded to 32 columns (cols 16-31 zero) so the M=32 context
            # matmuls initialize full 32-row PSUM strips.
            attnT_sb = work.tile([P, 4, 32], bf16)
            nc.gpsimd.memset(attnT_sb, 0.0)
            for j in range(4):
                ps_t = ps_sm.tile([P, BS], f32, tag="small16")
                nc.tensor.transpose(ps_t, attn_f32[:, j * P:(j + 1) * P], ident16)
                nc.vector.tensor_copy(attnT_sb[:, j, 0:BS], ps_t)

            # ---- context, wave-packed: 4 batches per [128, 512] PSUM ----
            # matmul with full [128,16] attnT lhsT writes 16 rows, of which
            # only row b (the true batch) is valid; packing 4 such blocks at
            # 32-aligned partition offsets (tile_position) puts the valid
            # rows at stride-33 partitions 33*k + 4*w, gathered by a single
            # strided DMA per wave.
            # Valid rows sit at psum partitions 33*b4 + 4*w; a host-built
            # 0/1 selection matrix (wblob blocks 16+w) gathers them into one
            # [16, 512] psum tile via matmul — avoiding per-row copies/DMAs
            # (compute engines can't address SBUF partitions off 0/32/64/96).
            # gather matmul for wave w is emitted after wave w+1's matmuls
            # (same in-order-PE stall avoidance as the energy matmuls).
            # Two gather groups (waves 0-1 and 2-3) so the first half of
            # ctx_out ships while the second half still computes.
            psum_gaths = [None, None]
            ctxgs = []

            def gather_mm(w):
                g = w // 2
                if w % 2 == 0:
                    psum_gaths[g] = ps_one.tile([8, E], f32, tag="energ", name=f"gath{g}")
                nc.tensor.matmul(
                    psum_gaths[g],
                    lhsT=wblob_sb[:, 16 + w, 0:8],
                    rhs=ctxgs[w],
                    start=(w % 2 == 0),
                    stop=(w % 2 == 1),
                )
                if w % 2 == 1:
                    gath = work.tile([8, E], f32, tag="gath", bufs=2)
                    nc.vector.tensor_copy(gath, psum_gaths[g])
                    nc.scalar.dma_start(ctx_out[g * 8:(g + 1) * 8, :], gath)

            for w in range(4):
                pc = ps_ctx.tile([P, E], f32, tag="ctx")
                for b4 in range(4):
                    b = w * 4 + b4
                    for j in range(4):
                        nc.tensor.matmul(
                            pc[b4 * 32:(b4 + 1) * 32, :],
                            lhsT=attnT_sb[:, j, :],
                            rhs=enc_nat_sb[:, 4 * b + j, :],
                            start=(j == 0),
                            stop=(j == 3),
                            tile_position=(0, b4 * 32),
                        )
                ctxg = work.tile([P, E], bf16, tag="ctxg", bufs=2)
                nc.vector.tensor_copy(ctxg, pc)
                ctxgs.append(ctxg)
                if w >= 1:
                    gather_mm(w - 1)
            gather_mm(3)

    return nc


def host_prepare(encoder_outputs, decoder_state, prev_attention_weights,
                 W_enc, W_dec, conv_w, W_loc, W_e, b_e):
    """Build per-core input maps (host-side marshaling, all numpy)."""
    f32 = np.float32
    enc = np.asarray(encoder_outputs, dtype=f32)
    dec = np.asarray(decoder_state, dtype=f32)
    prev = np.asarray(prev_attention_weights, dtype=f32)
    W_enc = np.asarray(W_enc, dtype=f32)
    W_dec = np.asarray(W_dec, dtype=f32)
    conv_w = np.asarray(conv_w, dtype=f32)
    W_loc = np.asarray(W_loc, dtype=f32)
    W_e = np.asarray(W_e, dtype=f32)

    # shared weight blob [128, 20, 128] (decT block 12 filled per core)
    wb = np.zeros((P, 20, 128), dtype=BF)
    wb[:, 0:4, :] = W_enc.T.reshape(4, P, A).transpose(1, 0, 2).astype(BF)
    wb[:, 4:12, :] = W_dec.T.reshape(8, P, A).transpose(1, 0, 2).astype(BF)
    Wcomb = W_loc @ conv_w[:, 0, :]                            # [A, KW]
    wb[0:KW, 13, :] = Wcomb.T.astype(BF)
    w_ediag = np.zeros((A, BS * BS), dtype=BF)
    we = W_e[0].astype(BF)                                     # [A]
    for b in range(BS):
        w_ediag[:, b * BS + b] = we
    wb[:, 14:16, :] = w_ediag.reshape(A, 2, 128)
    # context row-gather selection matrices: wave w contributes batches
    # 4w+b4 (valid psum partition 33*b4 + 4*w) to row 4*(w%2)+b4 of its
    # gather group's [8, E] psum
    for w in range(4):
        for b4 in range(4):
            wb[33 * b4 + 4 * w, 16 + w, 4 * (w % 2) + b4] = 1.0

    pp = np.pad(prev, ((0, 0), (15, 15)))                      # [B, T+30]

    in_maps = []
    for c in range(NCORES):
        sl = slice(c * BS, (c + 1) * BS)
        enc_c = enc[sl].reshape(BS * T, E)
        # partition-major natural layout: [p, chunk, e] in fp8 (context
        # path only; tolerable quantization, halves the enc_nat stream)
        enc_nat = np.ascontiguousarray(
            enc_c.reshape(BS * T // P, P, E).transpose(1, 0, 2)
        ).astype(ml_dtypes.float8_e4m3)
        # slice-major transposed layout: [p, slice, e_tile, bt_within]
        encT = np.ascontiguousarray(
            enc_c.T.reshape(4, P, 4, BS * T // 4).transpose(1, 2, 0, 3)
        ).astype(BF)
        rep = np.zeros((32, BS, T), dtype=ml_dtypes.float8_e4m3)
        pc = pp[sl]
        for k in range(KW):
            rep[k] = pc[:, k:k + T].astype(ml_dtypes.float8_e4m3)
        wb_c = wb.copy()
        wb_c[:, 12, :] = (
            dec[sl].T.reshape(8, P, BS).transpose(1, 0, 2).reshape(P, 128).astype(BF)
        )
        in_maps.append({
            "enc_nat": enc_nat,
            "encT": encT,
            "prevrep": np.ascontiguousarray(rep.reshape(32, BS * T)),
            "wblob": wb_c,
        })
    return in_maps


_NC_CACHE = {}


def get_nc():
    if "nc" not in _NC_CACHE:
        nc = bacc.Bacc("TRN2", debug=False, num_devices=NCORES)
        build_device_program(nc)
        nc.finalize()
        _NC_CACHE["nc"] = nc
    return _NC_CACHE["nc"]


def kernel(encoder_outputs, decoder_state, prev_attention_weights,
           W_enc, W_dec, conv_w, W_loc, W_e, b_e, _trace=False, _result_box=None):
    in_maps = host_prepare(
        encoder_outputs, decoder_state, prev_attention_weights,
        W_enc, W_dec, conv_w, W_loc, W_e, b_e,
    )
    nc = get_nc()
    res = bass_utils.run_bass_kernel_spmd(
        nc, in_maps, core_ids=list(range(NCORES)), trace=_trace,
    )
    if _result_box is not None:
        _result_box.append(res)
    ctx = np.concatenate([r["context_out"] for r in res.results], axis=0)
    attn = np.concatenate([r["attn_out"] for r in res.results], axis=0)
    return ctx.astype(np.float32), attn.astype(np.float32)


# revision 64
# speedup vs baseline: 1.0467x; 1.0467x over previous
"""Location-sensitive attention Trainium2 kernel (v2).

Strategy (data-parallel over batch, 8 cores, B=128 -> 16 per core):
  - Encoder shipped in BOTH layouts as bf16 (transposed [E, B*T] for the
    projection matmul, natural [B*T, E] for the context matmul); the
    17MB of encoder DMA at ~350GB/s is the critical path, everything
    else overlaps it.
  - All small weights packed into ONE partition-major DRAM blob so the
    weight load is a single large-descriptor DMA (the old per-weight
    strided DMAs burned ~10us in 64-512B descriptors).
  - encT DMAs sliced bt-major (2 proj tiles per DMA) so the projection
    chases the stream instead of waiting for whole e-chunks.
  - conv1d folded into W_loc on the host; loc_p accumulates into the
    same PSUM tile as the encoder projection via a host im2col.
  - decoder projection applied as a per-partition bias in the tanh.
  - energies via per-batch column-masked W_e ("diag" trick): all 16
    tiles accumulate into one [16, 512] PSUM tile (softmax layout).
  - context phase wave-packed: 8 batches per [128, 512] PSUM tile
    (matmul out at partition offset 16*k), one PSUM->SBUF copy and one
    stride-17 row-gather DMA per wave instead of 16 tiny copies+DMAs.
  - outputs go out on the second HWDGE ring (scalar) so they don't
    queue behind the enc_nat input stream on the sync ring.
  - b_e dropped: softmax is shift-invariant.
"""

import sys

for p in ("/opt/trn_rl_repo",):
    if p not in sys.path:
        sys.path.insert(0, p)

import numpy as np
import ml_dtypes

import concourse.bass as bass
import concourse.tile as tile
from concourse import mybir
from concourse import bacc
from concourse import bass_utils
from concourse.masks import make_identity

BF = ml_dtypes.bfloat16

NCORES = 8
B, T, E, D, A, F, KW = 128, 512, 512, 1024, 128, 32, 31
BS = B // NCORES          # 16 batches per core
NT = BS                   # 16 bt-tiles of 512 (tile i == batch i)
P = 128


def build_device_program(nc):
    dt = mybir.dt
    f32, bf16 = dt.float32, dt.bfloat16
    Act = mybir.ActivationFunctionType

    # Partition-major layouts so every DMA descriptor is a large
    # contiguous run.
    wblob = nc.dram_tensor("wblob", (P, 20, 128), bf16, kind="ExternalInput").ap()
    prevrep = nc.dram_tensor("prevrep", (32, BS * T), bf16, kind="ExternalInput").ap()
    # slice-major: [p, slice, et, bt_within] so each slice DMA is one
    # contiguous 16KB run per partition (fat descriptors, fast DGE).
    # Exactly 8 input DMAs total: Tile has 8 DMA-completion semaphore
    # lanes; more DMAs means lane sharing, which entangles projection
    # waits with enc_nat/output completions.
    encT = nc.dram_tensor("encT", (P, 4, 3, BS * T // 4), bf16, kind="ExternalInput").ap()
    # 4th e-chunk in fp8 (tolerable attn quantization, -1MB of stream)
    encT3 = nc.dram_tensor("encT3", (P, BS * T), dt.float8e4, kind="ExternalInput").ap()
    enc_nat = nc.dram_tensor("enc_nat", (P, BS * T // P, E), dt.float8e4, kind="ExternalInput").ap()
    ctx_out = nc.dram_tensor("context_out", (BS, E), f32, kind="ExternalOutput").ap()
    attn_out = nc.dram_tensor("attn_out", (BS, T), f32, kind="ExternalOutput").ap()

    with tile.TileContext(nc) as tc:
        with (
            tc.tile_pool(name="const", bufs=1) as const,
            tc.tile_pool(name="big", bufs=1) as big,
            tc.tile_pool(name="work", bufs=1) as work,
            tc.tile_pool(name="ps_pe", bufs=4, space="PSUM") as ps_pe,
            tc.tile_pool(name="ps_one", bufs=1, space="PSUM") as ps_one,
            tc.tile_pool(name="ps_sm", bufs=1, space="PSUM") as ps_sm,
            tc.tile_pool(name="ps_ctx", bufs=2, space="PSUM") as ps_ctx,
        ):
            # ---- DMA issue order on the sync ring = priority order ----
            wblob_sb = const.tile([P, 20, 128], bf16)
            nc.sync.dma_start(wblob_sb, wblob)
            encT3_sb = big.tile([P, BS * T], dt.float8e4)
            nc.sync.dma_start(encT3_sb, encT3)
            encT_sb = big.tile([P, 4, 3, BS * T // 4], bf16)
            prevrep_sb = const.tile([32, BS * T], bf16)
            for s in range(4):
                nc.sync.dma_start(encT_sb[:, s], encT[:, s])
                if s == 0:
                    # prevrep is first needed by tile 0's conv matmul, just
                    # after encT slice 0 — slot it behind slice 0
                    nc.sync.dma_start(prevrep_sb, prevrep)
            enc_nat_sb = big.tile([P, BS * T // P, E], dt.float8e4)
            for h in range(2):
                nc.sync.dma_start(
                    enc_nat_sb[:, h * 32:(h + 1) * 32, :], enc_nat[:, h * 32:(h + 1) * 32, :]
                )

            ident16 = const.tile([16, 16], f32)
            make_identity(nc, ident16)

            # ---- PE warm-up: the HAM clock gate only un-throttles the PE
            # (1.2 -> 2.4 GHz) after ~3.4us of sustained full-array matmul
            # activity. The real matmul stream is DMA-paced with gaps, so
            # it never warms on its own. Run full-width dummy matmuls on a
            # zeroed tile while the PE waits for the first encT slice.
            dummy_sb = const.tile([P, T], bf16)
            nc.gpsimd.memset(dummy_sb, 0.0)
            warm_ps = ps_ctx.tile([P, T], f32, tag="ctx")
            for wu in range(52):
                nc.tensor.matmul(
                    warm_ps, lhsT=dummy_sb[:, 0:P], rhs=dummy_sb,
                    start=True, stop=True,
                )
            # pre-load the scalar engine's tanh/exp tables now, so the
            # table-load DMAs don't interleave with the encoder stream
            actwarm = work.tile([16, 16], bf16)
            nc.scalar.activation(actwarm, dummy_sb[0:16, 0:16], Act.Tanh)
            actwarm2 = work.tile([16, 16], bf16)
            nc.scalar.activation(
                actwarm2, dummy_sb[0:16, 0:16], Act.Exp, accum_out=None
            )

            # ---- projection + tanh + energies (chases the encT stream) ----
            # The energy matmul for tile i is emitted after tile i+1's
            # projection matmuls: the PE queue is in-order, so putting it
            # right after tile i's would stall the PE on tanh_i (scalar).
            psum_energ = ps_one.tile([BS, T], f32, tag="energ")
            tanh_tiles = []

            def energy_mm(i):
                eblk, ecol = 14 + i // 8, (i % 8) * 16
                nc.tensor.matmul(
                    psum_energ,
                    lhsT=wblob_sb[:, eblk, ecol:ecol + 16],
                    rhs=tanh_tiles[i],
                    start=(i == 0),
                    stop=(i == NT - 1),
                )

            decp_sb = const.tile([P, BS], f32)
            for i in range(NT):  # tile i == batch i
                pe_t = ps_pe.tile([A, T], f32, tag="pe")
                s, hh = i // 4, (i % 4) * T
                for et in range(3):
                    nc.tensor.matmul(
                        pe_t,
                        lhsT=wblob_sb[:, et, :],
                        rhs=encT_sb[:, s, et, hh:hh + T],
                        start=(et == 0),
                        stop=False,
                    )
                nc.tensor.matmul(
                    pe_t,
                    lhsT=wblob_sb[:, 3, :],
                    rhs=encT3_sb[:, i * T:(i + 1) * T],
                    start=False,
                    stop=False,
                )
                nc.tensor.matmul(
                    pe_t,
                    lhsT=wblob_sb[0:32, 13, :],
                    rhs=prevrep_sb[:, i * T:(i + 1) * T],
                    start=False,
                    stop=True,
                )
                if i == 0:
                    # dec_p (tanh bias): emitted after tile 0's projection
                    # so its wblob wait doesn't delay the first real matmul
                    psum_dec = ps_sm.tile([P, BS], f32, tag="small16")
                    for dti in range(8):
                        nc.tensor.matmul(
                            psum_dec,
                            lhsT=wblob_sb[:, 4 + dti, :],
                            rhs=wblob_sb[:, 12, dti * 16:(dti + 1) * 16],
                            start=(dti == 0),
                            stop=(dti == 7),
                        )
                    nc.vector.tensor_copy(decp_sb, psum_dec)
                tanh_t = work.tile([A, T], bf16, tag="tanh", bufs=4)
                nc.scalar.activation(
                    tanh_t, pe_t, Act.Tanh, bias=decp_sb[:, i:i + 1], scale=1.0
                )
                tanh_tiles.append(tanh_t)
                if i >= 1:
                    energy_mm(i - 1)
            energy_mm(NT - 1)

            # keep the PE's HAM clock-gate warm across the softmax gap
            warm2 = ps_pe.tile([P, T], f32, tag="pe")
            for wu in range(6):
                nc.tensor.matmul(
                    warm2, lhsT=dummy_sb[:, 0:P], rhs=dummy_sb,
                    start=True, stop=True,
                )

            # ---- softmax over T (psum_energ is [16, 512]) ----
            # no max-subtraction: |energies| <= ~9 (W_e is 1/sqrt(A)-scaled,
            # tanh in [-1,1]), so f32 exp cannot overflow
            attn_exp = work.tile([BS, T], f32)
            esum = work.tile([BS, 1], f32)
            nc.scalar.activation(
                attn_exp, psum_energ, Act.Exp, scale=1.0, accum_out=esum
            )
            rs = work.tile([BS, 1], f32)
            nc.vector.reciprocal(rs, esum)
            attn_f32 = work.tile([BS, T], f32)
            nc.vector.tensor_scalar_mul(attn_f32, attn_exp, rs)
            # output on the scalar HWDGE ring: doesn't queue behind enc_nat
            nc.scalar.dma_start(attn_out, attn_f32)

            # ---- transpose attn -> [t, b] columns ----
            # padded to 32 columns (cols 16-31 zero) so the M=32 context
            # matmuls initialize full 32-row PSUM strips.
            attnT_sb = work.tile([P, 4, 32], bf16)
            nc.gpsimd.memset(attnT_sb, 0.0)
            for j in range(4):
                ps_t = ps_sm.tile([P, BS], f32, tag="small16")
                nc.tensor.transpose(ps_t, attn_f32[:, j * P:(j + 1) * P], ident16)
                nc.vector.tensor_copy(attnT_sb[:, j, 0:BS], ps_t)

            # ---- context, wave-packed: 4 batches per [128, 512] PSUM ----
            # matmul with full [128,16] attnT lhsT writes 16 rows, of which
            # only row b (the true batch) is valid; packing 4 such blocks at
            # 32-aligned partition offsets (tile_position) puts the valid
            # rows at stride-33 partitions 33*k + 4*w, gathered by a single
            # strided DMA per wave.
            # Valid rows sit at psum partitions 33*b4 + 4*w; a host-built
            # 0/1 selection matrix (wblob blocks 16+w) gathers them into one
            # [16, 512] psum tile via matmul — avoiding per-row copies/DMAs
            # (compute engines can't address SBUF partitions off 0/32/64/96).
            # gather matmul for wave w is emitted after wave w+1's matmuls
            # (same in-order-PE stall avoidance as the energy matmuls).
            # Two gather groups (waves 0-1 and 2-3) so the first half of
            # ctx_out ships while the second half still computes.
            psum_gaths = [None, None]
            ctxgs = []

            def gather_mm(w):
                g = w // 2
                if w % 2 == 0:
                    psum_gaths[g] = ps_one.tile([8, E], f32, tag="energ", name=f"gath{g}")
                nc.tensor.matmul(
                    psum_gaths[g],
                    lhsT=wblob_sb[:, 16 + w, 0:8],
                    rhs=ctxgs[w],
                    start=(w % 2 == 0),
                    stop=(w % 2 == 1),
                )
                if w % 2 == 1:
                    gath = work.tile([8, E], f32, tag="gath", bufs=2)
                    nc.vector.tensor_copy(gath, psum_gaths[g])
                    nc.scalar.dma_start(ctx_out[g * 8:(g + 1) * 8, :], gath)

            for w in range(4):
                pc = ps_ctx.tile([P, E], f32, tag="ctx")
                for b4 in range(4):
                    b = w * 4 + b4
                    for j in range(4):
                        nc.tensor.matmul(
                            pc[b4 * 32:(b4 + 1) * 32, :],
                            lhsT=attnT_sb[:, j, :],
                            rhs=enc_nat_sb[:, 4 * b + j, :],
                            start=(j == 0),
                            stop=(j == 3),
                            tile_position=(0, b4 * 32),
                        )
                ctxg = work.tile([P, E], bf16, tag="ctxg", bufs=2)
                nc.vector.tensor_copy(ctxg, pc)
                ctxgs.append(ctxg)
                if w >= 1:
                    gather_mm(w - 1)
            gather_mm(3)

    return nc


def host_prepare(encoder_outputs, decoder_state, prev_attention_weights,
                 W_enc, W_dec, conv_w, W_loc, W_e, b_e):
    """Build per-core input maps (host-side marshaling, all numpy)."""
    f32 = np.float32
    enc = np.asarray(encoder_outputs, dtype=f32)
    dec = np.asarray(decoder_state, dtype=f32)
    prev = np.asarray(prev_attention_weights, dtype=f32)
    W_enc = np.asarray(W_enc, dtype=f32)
    W_dec = np.asarray(W_dec, dtype=f32)
    conv_w = np.asarray(conv_w, dtype=f32)
    W_loc = np.asarray(W_loc, dtype=f32)
    W_e = np.asarray(W_e, dtype=f32)

    # shared weight blob [128, 20, 128] (decT block 12 filled per core)
    wb = np.zeros((P, 20, 128), dtype=BF)
    wb[:, 0:4, :] = W_enc.T.reshape(4, P, A).transpose(1, 0, 2).astype(BF)
    wb[:, 4:12, :] = W_dec.T.reshape(8, P, A).transpose(1, 0, 2).astype(BF)
    Wcomb = W_loc @ conv_w[:, 0, :]                            # [A, KW]
    wb[0:KW, 13, :] = Wcomb.T.astype(BF)
    w_ediag = np.zeros((A, BS * BS), dtype=BF)
    we = W_e[0].astype(BF)                                     # [A]
    for b in range(BS):
        w_ediag[:, b * BS + b] = we
    wb[:, 14:16, :] = w_ediag.reshape(A, 2, 128)
    # context row-gather selection matrices: wave w contributes batches
    # 4w+b4 (valid psum partition 33*b4 + 4*w) to row 4*(w%2)+b4 of its
    # gather group's [8, E] psum
    for w in range(4):
        for b4 in range(4):
            wb[33 * b4 + 4 * w, 16 + w, 4 * (w % 2) + b4] = 1.0

    pp = np.pad(prev, ((0, 0), (15, 15)))                      # [B, T+30]

    in_maps = []
    for c in range(NCORES):
        sl = slice(c * BS, (c + 1) * BS)
        enc_c = enc[sl].reshape(BS * T, E)
        # partition-major natural layout: [p, chunk, e] in fp8 (context
        # path only; tolerable quantization, halves the enc_nat stream)
        enc_nat = np.ascontiguousarray(
            enc_c.reshape(BS * T // P, P, E).transpose(1, 0, 2)
        ).astype(ml_dtypes.float8_e4m3)
        # slice-major transposed layout: [p, slice, e_tile, bt_within]
        # (e-chunks 0-2 bf16; chunk 3 separate in fp8)
        encT = np.ascontiguousarray(
            enc_c.T[0:384].reshape(3, P, 4, BS * T // 4).transpose(1, 2, 0, 3)
        ).astype(BF)
        encT3 = np.ascontiguousarray(enc_c.T[384:512]).astype(ml_dtypes.float8_e4m3)
        rep = np.zeros((32, BS, T), dtype=BF)
        pc = pp[sl]
        for k in range(KW):
            rep[k] = pc[:, k:k + T].astype(BF)
        wb_c = wb.copy()
        wb_c[:, 12, :] = (
            dec[sl].T.reshape(8, P, BS).transpose(1, 0, 2).reshape(P, 128).astype(BF)
        )
        in_maps.append({
            "enc_nat": enc_nat,
            "encT": encT,
            "encT3": encT3,
            "prevrep": np.ascontiguousarray(rep.reshape(32, BS * T)),
            "wblob": wb_c,
        })
    return in_maps


_NC_CACHE = {}


def get_nc():
    if "nc" not in _NC_CACHE:
        nc = bacc.Bacc("TRN2", debug=False, num_devices=NCORES)
        build_device_program(nc)
        nc.finalize()
        _NC_CACHE["nc"] = nc
    return _NC_CACHE["nc"]


def kernel(encoder_outputs, decoder_state, prev_attention_weights,
           W_enc, W_dec, conv_w, W_loc, W_e, b_e, _trace=False, _result_box=None):
    in_maps = host_prepare(
        encoder_outputs, decoder_state, prev_attention_weights,
        W_enc, W_dec, conv_w, W_loc, W_e, b_e,
    )
    nc = get_nc()
    res = bass_utils.run_bass_kernel_spmd(
        nc, in_maps, core_ids=list(range(NCORES)), trace=_trace,
    )
    if _result_box is not None:
        _result_box.append(res)
    ctx = np.concatenate([r["context_out"] for r in res.results], axis=0)
    attn = np.concatenate([r["attn_out"] for r in res.results], axis=0)
    return ctx.astype(np.float32), attn.astype(np.float32)
